# revision 1
# baseline (speedup 1.0000x reference)
"""GAT head (DGAT) Trainium2 kernel: 8-core row-sharded masked-softmax attention.

Math (per reference):
  h = X @ W                       [N, 64]
  e = leaky_relu(src_i + dst_j, 0.2), src = h@a[:64], dst = h@a[64:]
  att = softmax(where(adj>0, e, -9e15), axis=1)
  out = elu(att @ h)

Kernel strategy per core c (rows R = [1024c, 1024c+1024)):
  - DMA-cast adj slab int32 -> bf16 {0,1} (SWDGE cast during DMA)
  - PE: u[j, i] = BIG*adjT (transpose-matmul vs BIG*I, bf16)
                + src_i     (K=1 fp32 rank-1 matmul, ones x src row)
  - ACT: e = prelu(u + (dst_j - BIG), alpha=0.2)   [masked: ~0.2*(s-BIG) -> exp -> 0]
         p = exp(e)
  - PE: agg[65, i] += hext_jc^T @ p_jc  (hext = [h | 1]; row 64 = softmax denom)
  - finalize: transpose back, scale by 1/denom, ELU, DMA out.
"""
import os
import sys
import numpy as np

sys.path.insert(0, "/opt/trn_rl_repo")

import concourse.bass as bass
import concourse.bacc as bacc
import concourse.tile as tile
from concourse import mybir
from concourse.masks import make_identity
from concourse import bass_utils

P = 128
N = 8192
DIN = 256
DOUT = 64
NCORES = 8
R = N // NCORES          # rows per core
BIG = 1024.0
ALPHA = 0.2
JT_W = 2048              # j supertile width (dev-tunable)
def _jt():
    return JT_W, N // JT_W, JT_W // P
NJC = N // P             # 64
NIC = N // P             # 64 chunks for h prep
SUBS = R // P            # 8 row sub-blocks per core
F32 = mybir.dt.float32
F16 = mybir.dt.float16
BF16 = mybir.dt.bfloat16
I32 = mybir.dt.int32

_cached = {}
ABLATE = set()   # dev-only: {'act','transp','srcadd','agg','dma'}
UBUFS = 2
ADJ_BUFS = 20
ACT1_SPLIT = False
AGG_DELAY = 1
PBUFS = 6
DVE_LEAKY = True
JT_FRACTION = 0


def build_module(rep=1, rep_loop=1):
    key = ("nc", rep, rep_loop)
    if key in _cached:
        return _cached[key]
    nc = bacc.Bacc("TRN2", target_bir_lowering=False, debug=False, num_devices=NCORES)

    adj_d = nc.dram_tensor("adjslab", [R, N], I32, kind="ExternalInput").ap()
    x_d = nc.dram_tensor("xt", [DIN, N], F32, kind="ExternalInput").ap()
    w_d = nc.dram_tensor("w", [DIN, DOUT], F32, kind="ExternalInput").ap()
    a_d = nc.dram_tensor("av", [2 * DOUT, 1], F32, kind="ExternalInput").ap()
    out_d = nc.dram_tensor("out", [R, DOUT], F32, kind="ExternalOutput").ap()

    with tile.TileContext(nc) as tc:
        for _ in range(rep):
            _build(nc, tc, adj_d, x_d, w_d, a_d, out_d, rep_loop)

    nc.compile()
    _cached[key] = nc
    return nc


def _build(nc, tc, adj_d, x_d, w_d, a_d, out_d, rep_loop=1):
    from contextlib import ExitStack

    with ExitStack() as ctx:
        const = ctx.enter_context(tc.tile_pool(name="const", bufs=1))

        # ---- constants ----
        bigI = const.tile([P, P], BF16)
        make_identity(nc, bigI)
        nc.vector.tensor_scalar_mul(bigI, bigI, BIG)
        idf = const.tile([P, P], F32)
        make_identity(nc, idf)
        ones2 = const.tile([2, P], BF16)
        nc.vector.memset(ones2, 1.0)
        alpha_t = const.tile([P, 1], F32)
        nc.vector.memset(alpha_t, ALPHA)

        w_a = const.tile([P, DOUT], F32)
        w_b = const.tile([P, DOUT], F32)
        nc.sync.dma_start(out=w_a, in_=w_d[0:P, :])
        nc.sync.dma_start(out=w_b, in_=w_d[P:DIN, :])
        a1 = const.tile([DOUT, 1], F32)
        a2 = const.tile([DOUT, 1], F32)
        nc.sync.dma_start(out=a1, in_=a_d[0:DOUT, :])
        nc.sync.dma_start(out=a2, in_=a_d[DOUT : 2 * DOUT, :])

        # persistent per-core data
        hext_c = [const.tile([P, 65], BF16, tag=f"hx{c}", name=f"hx{c}")
                  for c in range(NJC)]          # [j%P, f|one] per j-chunk
        dstb8 = [const.tile([P, 8], F32, tag=f"db{b}", name=f"db{b}")
                 for b in range(NJC // 8)]      # dst - BIG, batches of 8 chunks
        src_my = const.tile([1, R], F32)
        src_hi = const.tile([1, R], BF16)
        src_lo = const.tile([1, R], BF16)

        # ---- stage A: h = X@W (from pre-transposed X), hT, src, dst ----
        # Ordered for earliest stage-B unblock: chunked X loads -> hT (f32r,
        # fast) -> src -> dst -> h/hext chunks (streamed, consumed lazily by
        # the deferred aggregation matmuls).
        XCH = 4                 # x chunk tiles per half
        XW = N // XCH           # 2048 cols per chunk
        with tc.tile_pool(name="prep", bufs=1) as prep, \
             tc.tile_pool(name="prep_ps", bufs=2, space="PSUM") as prep_ps:
            xt_t = [prep.tile([P, XW], F32, tag=f"xta{k}", name=f"xta{k}")
                    for k in range(XCH)]
            xt_b = [prep.tile([P, XW], F32, tag=f"xtb{k}", name=f"xtb{k}")
                    for k in range(XCH)]
            ht_sb = prep.tile([DOUT, N], F32)   # h^T


            pid = nc.partition_id()

            for k in range(XCH):
                nc.sync.dma_start(out=xt_t[k], in_=x_d[0:P, k * XW : (k + 1) * XW])
                nc.sync.dma_start(out=xt_b[k], in_=x_d[P:DIN, k * XW : (k + 1) * XW])
            # hT [64, N] via f32r (1 cyc/col at >=256 free), dst batches
            # interleaved so dstb8[b] unblocks ACT1 as early as possible.
            for m in range(N // 512):
                k, off = m // (XW // 512), (m % (XW // 512)) * 512
                ht_ps = prep_ps.tile([DOUT, 512], F32, tag="pp", name="ht_ps")
                nc.tensor.matmul(ht_ps, lhsT=w_a,
                                 rhs=xt_t[k][:, off : off + 512],
                                 start=True, stop=False)
                nc.tensor.matmul(ht_ps, lhsT=w_b,
                                 rhs=xt_b[k][:, off : off + 512],
                                 start=False, stop=True)
                if m % 2 == 0:
                    nc.vector.tensor_copy(ht_sb[:, m * 512 : (m + 1) * 512], ht_ps)
                else:
                    nc.scalar.copy(ht_sb[:, m * 512 : (m + 1) * 512], ht_ps)
                if m % 2 == 1:
                    b = (m - 1) // 2
                    d_ps = prep_ps.tile([P, 8], F32, tag="pp", name="d_ps")
                    for bb in range(8):
                        c = b * 8 + bb
                        nc.tensor.matmul(d_ps[:, bb : bb + 1],
                                         lhsT=ht_sb[:, c * P : (c + 1) * P], rhs=a2,
                                         start=True, stop=True)
                    nc.vector.tensor_scalar_add(dstb8[b], d_ps, -BIG)
                    # h chunks -> hext tiles [128 i, 64] (+ ones col)
                    for c in range(b * 8, (b + 1) * 8):
                        kk, off2 = c // (XW // P), (c % (XW // P)) * P
                        h_ps = prep_ps.tile([P, DOUT], F32, tag="pp", name="h_ps")
                        nc.tensor.matmul(h_ps, lhsT=xt_t[kk][:, off2 : off2 + P],
                                         rhs=w_a, start=True, stop=False)
                        nc.tensor.matmul(h_ps, lhsT=xt_b[kk][:, off2 : off2 + P],
                                         rhs=w_b, start=False, stop=True)
                        if c % 2 == 0:
                            nc.scalar.copy(hext_c[c][:, 0:DOUT], h_ps)
                        else:
                            nc.vector.tensor_copy(hext_c[c][:, 0:DOUT], h_ps)
                        nc.vector.memset(hext_c[c][:, DOUT : DOUT + 1], 1.0)

            # src for this core's rows (dynamic SBUF slice by partition id)
            for ib in range(2):
                s_ps = prep_ps.tile([1, 512], F32, tag="pp", name="s_ps")
                nc.tensor.matmul(
                    s_ps, lhsT=a1,
                    rhs=ht_sb[0:DOUT, bass.ds(pid * R + ib * 512, 512)],
                    start=True, stop=True)
                nc.vector.tensor_copy(src_my[:, ib * 512 : (ib + 1) * 512], s_ps)
            nc.vector.tensor_copy(src_hi, src_my)
            nc.vector.tensor_tensor(out=src_lo, in0=src_my, in1=src_hi,
                                    op=mybir.AluOpType.subtract)

        # ---- stage B: main attention loop ----
        adjf_pool = ctx.enter_context(tc.tile_pool(name="adjf", bufs=ADJ_BUFS))
        agg_pool = ctx.enter_context(tc.tile_pool(name="agg_ps", bufs=2, space="PSUM"))
        e_pool = ctx.enter_context(tc.tile_pool(name="e_sb", bufs=4))
        p_pool = ctx.enter_context(tc.tile_pool(name="p_sb", bufs=PBUFS))

        agg = [agg_pool.tile([65, 512], F32, tag=f"agg{ib}", name=f"agg{ib}", bufs=1)
               for ib in range(2)]

        from contextlib import nullcontext
        with tc.tile_pool(name="u_ps", bufs=UBUFS, space="PSUM") as u_pool:
            loop_cm = tc.For_i(0, rep_loop, 1) if rep_loop > 1 else nullcontext()
            with loop_cm:
                _stageB(nc, tc, adj_d, adjf_pool, u_pool, e_pool, p_pool,
                        agg, bigI, ones2, (src_hi, src_lo), dstb8, alpha_t, hext_c)

        # ---- finalize ----
        with tc.tile_pool(name="fin", bufs=4) as fin, \
             tc.tile_pool(name="fin_ps", bufs=2, space="PSUM") as fin_ps:
            for ib in range(2):
                agg_sb = fin.tile([65, 512], F32, tag="agg_sb")
                nc.vector.tensor_copy(agg_sb, agg[ib])
                for q in range(4):
                    o_ps = fin_ps.tile([P, 65], F32, tag="o_ps")
                    nc.tensor.matmul(o_ps, lhsT=agg_sb[:, q * P : (q + 1) * P],
                                     rhs=idf[0:65, 0:65], start=True, stop=True)
                    rc = fin.tile([P, 1], F32, tag="rc")
                    nc.vector.reciprocal(rc, o_ps[:, DOUT : DOUT + 1])
                    hp = fin.tile([P, DOUT], F32, tag="hp")
                    nc.vector.tensor_scalar_mul(hp, o_ps[:, 0:DOUT], rc)
                    # elu = max(x,0) + exp(min(x,0)) - 1
                    ng = fin.tile([P, DOUT], F32, tag="ng")
                    nc.vector.tensor_scalar_min(ng, hp, 0.0)
                    ex = fin.tile([P, DOUT], F32, tag="ex")
                    nc.scalar.activation(ex, ng, mybir.ActivationFunctionType.Exp)
                    ps_ = fin.tile([P, DOUT], F32, tag="ps_")
                    nc.vector.tensor_scalar_max(ps_, hp, 0.0)
                    ob = fin.tile([P, DOUT], F32, tag="ob")
                    nc.vector.tensor_tensor(out=ob, in0=ex, in1=ps_,
                                            op=mybir.AluOpType.add)
                    nc.vector.tensor_scalar_add(ob, ob, -1.0)
                    g = ib * 4 + q
                    nc.sync.dma_start(out=out_d[g * P : (g + 1) * P, :], in_=ob)


def kernel(**inputs) -> np.ndarray:
    xt = np.ascontiguousarray(np.asarray(inputs["input"], np.float32)[0].T)
    adj = np.ascontiguousarray(np.asarray(inputs["adj"], np.int32))
    w = np.ascontiguousarray(np.asarray(inputs["w"], np.float32))
    a = np.ascontiguousarray(np.asarray(inputs["a"], np.float32).reshape(2 * DOUT, 1))

    nc = build_module()
    in_maps = []
    for c in range(NCORES):
        in_maps.append({
            "adjslab": adj[c * R : (c + 1) * R, :],
            "xt": xt,
            "w": w,
            "av": a,
        })
    res = bass_utils.run_bass_kernel_spmd(nc, in_maps, core_ids=list(range(NCORES)))
    out = np.concatenate([res.results[c]["out"] for c in range(NCORES)], axis=0)
    return out.astype(np.float32)


if __name__ == "__main__":
    rng = np.random.default_rng(0)
    ins = {
        "input": rng.standard_normal((1, N, DIN)).astype(np.float32),
        "adj": rng.integers(0, 2, size=(N, N)).astype(np.int32),
        "w": rng.standard_normal((DIN, DOUT)).astype(np.float32) * 0.1,
        "a": rng.standard_normal((2 * DOUT, 1)).astype(np.float32) * 0.1,
    }
    o = kernel(**ins)
    print("kernel out", o.shape, o.dtype)


def _stageB(nc, tc, adj_d, adjf_pool, u_pool, e_pool, p_pool,
            agg, bigI, ones2, src2, dstb8, alpha_t, hext_c):
        JTW, NJT, JCPJT = _jt()
        ebig = None
        pending = []

        def emit_agg(pbig_, jc_pair):
            gw = 32 if "agg" in ABLATE else 512
            for half, jcx in ((0, jc_pair), (1, jc_pair + 1)):
                for ib in range(2):
                    nc.tensor.matmul(
                        agg[ib][:, 0:gw],
                        lhsT=hext_c[jcx],
                        rhs=pbig_[:, half * 1024 + ib * 512 : half * 1024 + ib * 512 + gw],
                        start=(jcx == 0),
                        stop=(jcx == NJC - 1),
                    )

        for jt in range(NJT if not JT_FRACTION else max(1, NJT // JT_FRACTION)):
            adjf = []
            for s in range(SUBS):
                t = adjf_pool.tile([P, JTW], BF16, tag="adjf")
                if "dma" in ABLATE:
                    if s == 0:
                        nc.gpsimd.dma_start(out=t[:, 0:32],
                            in_=adj_d[s * P : (s + 1) * P, jt * JTW : jt * JTW + 32])
                    else:
                        nc.vector.memset(t[:, 0:32], 0.0)
                else:
                    nc.gpsimd.dma_start(
                        out=t,
                        in_=adj_d[s * P : (s + 1) * P, jt * JTW : (jt + 1) * JTW],
                    )
                adjf.append(t)
            for k in range(JCPJT):
                jc = jt * JCPJT + k
                u = u_pool.tile([P, 1024], F32, tag="u")
                tw = 16 if "transp" in ABLATE else P
                for q in range(8):
                    ib, qq = q // 4, q % 4
                    nc.tensor.matmul(
                        u[:, ib * 512 + qq * P : ib * 512 + qq * P + tw],
                        lhsT=adjf[q][:, k * P : (k + 1) * P],
                        rhs=bigI[:, 0:tw],
                        start=(qq == 0),
                        stop=False,
                    )
                sw = 16 if "srcadd" in ABLATE else 512
                for ib in range(2):
                    nc.tensor.matmul(
                        u[:, ib * 512 : ib * 512 + sw],
                        lhsT=ones2[0:1, :],
                        rhs=src2[0][:, ib * 512 : ib * 512 + sw],
                        start=False,
                        stop=False,
                    )
                    nc.tensor.matmul(
                        u[:, ib * 512 : ib * 512 + sw],
                        lhsT=ones2[0:1, :],
                        rhs=src2[1][:, ib * 512 : ib * 512 + sw],
                        start=False,
                        stop=True,
                    )
                if jc % 2 == 0:
                    ebig = e_pool.tile([P, 2048], F32, tag="ebig")
                aw = 32 if "act" in ABLATE else 1024
                if ACT1_SPLIT and aw == 1024:
                    for hb in range(2):
                        nc.scalar.activation(
                            ebig[:, (jc % 2) * 1024 + hb * 512 : (jc % 2) * 1024 + (hb + 1) * 512],
                            u[:, hb * 512 : (hb + 1) * 512],
                            mybir.ActivationFunctionType.Prelu,
                            bias=dstb8[jc // 8][:, jc % 8 : jc % 8 + 1],
                            scale=1.0,
                            alpha=alpha_t,
                        )
                elif DVE_LEAKY and jc % 2 == 1:
                    eb = ebig[:, 1024 : 1024 + aw]
                    tmp = e_pool.tile([P, 1024], F32, tag="lk", name="lk")
                    nc.vector.tensor_scalar(
                        out=eb, in0=u[:, 0:aw],
                        scalar1=dstb8[jc // 8][:, jc % 8 : jc % 8 + 1],
                        scalar2=None, op0=mybir.AluOpType.add)
                    nc.vector.tensor_scalar_mul(tmp[:, 0:aw], eb, 0.2)
                    nc.vector.tensor_tensor(out=eb, in0=eb, in1=tmp[:, 0:aw],
                                            op=mybir.AluOpType.max)
                else:
                    nc.scalar.activation(
                        ebig[:, (jc % 2) * 1024 : (jc % 2) * 1024 + aw],
                        u[:, 0:aw],
                        mybir.ActivationFunctionType.Prelu,
                        bias=dstb8[jc // 8][:, jc % 8 : jc % 8 + 1],
                        scale=1.0,
                        alpha=alpha_t,
                    )
                if jc % 2 == 1:
                    pbig = p_pool.tile([P, 2048], BF16, tag="pbig")
                    pw = 32 if "act" in ABLATE else 2048
                    nc.scalar.activation(
                        pbig[:, 0:pw], ebig[:, 0:pw], mybir.ActivationFunctionType.Exp
                    )
                    pending.append((pbig, jc - 1))
                    if len(pending) > AGG_DELAY:
                        emit_agg(*pending.pop(0))
        while pending:
            emit_agg(*pending.pop(0))



# revision 7
# speedup vs baseline: 1.3583x; 1.3583x over previous
"""GAT head (DGAT) Trainium2 kernel: 8-core row-sharded masked-softmax attention.

Math (per reference):
  h = X @ W                       [N, 64]
  e = leaky_relu(src_i + dst_j, 0.2), src = h@a[:64], dst = h@a[64:]
  att = softmax(where(adj>0, e, -9e15), axis=1)
  out = elu(att @ h)

Select-factorization strategy (no per-element exp!):
  exp(leaky(s)) = C*e^src*e^dst + (1-C)*e^{0.2 src}*e^{0.2 dst},  C=[s>0]
so each branch is rank-1 in (i,j). Per core c (rows R = [1024c, 1024c+1024),
tiles are [128 j-part, 1024 i-free], adjacency comes HOST-TRANSPOSED as int8):
  - DVE prefills V = src_i/L + dst_j/L - 1  (fused tensor_scalar, fp16, 4x)
  - DMA-accumulates adjT onto V (int8->fp16 cast + add) giving
      t = adjT + s/L - 1   (t>0 iff adj&s>0; t>=-.5 iff adj)
  - M1 = is_gt(t,0) (fp16 {0,1}, one 4x DVE op)
  - D-tiles: adjr = is_ge(t,-.5); M2 = adjr - M1; PE aggs:
      agg1 += hext1^T@M1, agg2 += hext2^T@M2
  - P-tiles: PE aggs: agg1 += hext1^T@M1, agg1b += hext2^T@M1,
      agg2 += hext2^T@t  (affine part removed at finalize via rank-1
      correction G (x) srcL + H (x) 1, G/H computed in stage A)
  hext1 = [e^{dst-M}*h | e^{dst-M}], hext2 = [e^{.2(dst-M)}*h | e^{.2(dst-M)}]
  finalize: num/den = sigma_i*agg1 + (agg2 - corr - agg1b), sigma =
  exp(.8(src_i+M)); out = elu(num/den).
"""
import os
import sys
import numpy as np

sys.path.insert(0, "/opt/trn_rl_repo")

import concourse.bass as bass
import concourse.bacc as bacc
import concourse.tile as tile
from concourse import mybir
from concourse.masks import make_identity
from concourse import bass_utils

P = 128
N = 8192
DIN = 256
DOUT = 64
NCORES = 8
R = N // NCORES          # rows per core
L = 128.0                # s-scale; needs max|src+dst| < L/2
ALPHA = 0.2
NJC = N // P             # 64 j-chunks
F32 = mybir.dt.float32
F16 = mybir.dt.float16
BF16 = mybir.dt.bfloat16
I8 = mybir.dt.int8

_cached = {}
D_MOD = 15
D_THR = 7               # jc % D_MOD < D_THR -> D-path (explicit M2)
M2_POOL_MOD = 0         # if >0, every k-th D-tile's M2 subtract runs on gpsimd
VBUFS = 6
MBUFS = 6
XBUFS = 4


def _is_d(jc):
    return (jc % D_MOD) < D_THR


def build_module(rep=1, rep_loop=1):
    key = ("nc", rep, rep_loop)
    if key in _cached:
        return _cached[key]
    nc = bacc.Bacc("TRN2", target_bir_lowering=False, debug=False, num_devices=NCORES)

    adjT_d = nc.dram_tensor("adjT", [N, R], I8, kind="ExternalInput").ap()
    x_d = nc.dram_tensor("xt", [DIN, N], BF16, kind="ExternalInput").ap()
    w_d = nc.dram_tensor("w", [DIN, DOUT], BF16, kind="ExternalInput").ap()
    a_d = nc.dram_tensor("av", [2 * DOUT, 1], F32, kind="ExternalInput").ap()
    out_d = nc.dram_tensor("out", [R, DOUT], F32, kind="ExternalOutput").ap()

    with tile.TileContext(nc) as tc:
        for _ in range(rep):
            _build(nc, tc, adjT_d, x_d, w_d, a_d, out_d, rep_loop)

    nc.compile()
    _cached[key] = nc
    return nc


def _build(nc, tc, adjT_d, x_d, w_d, a_d, out_d, rep_loop=1):
    from contextlib import ExitStack, nullcontext

    pjcs = [jc for jc in range(NJC) if not _is_d(jc)]

    with ExitStack() as ctx:
        const = ctx.enter_context(tc.tile_pool(name="const", bufs=1))

        idf = const.tile([P, P], F32)
        make_identity(nc, idf)
        ones1 = const.tile([1, P], F32)
        nc.vector.memset(ones1, 1.0)
        onescol = const.tile([P, 1], F16)
        nc.vector.memset(onescol, 1.0)

        w_a = const.tile([P, DOUT], BF16)
        w_b = const.tile([P, DOUT], BF16)
        nc.sync.dma_start(out=w_a, in_=w_d[0:P, :])
        nc.sync.dma_start(out=w_b, in_=w_d[P:DIN, :])
        a1 = const.tile([DOUT, 1], F32)
        a2 = const.tile([DOUT, 1], F32)
        nc.sync.dma_start(out=a1, in_=a_d[0:DOUT, :])
        nc.sync.dma_start(out=a2, in_=a_d[DOUT : 2 * DOUT, :])

        # persistent per-core data
        hext1_c = [const.tile([P, 65], F16, tag=f"h1_{c}", name=f"h1_{c}")
                   for c in range(NJC)]
        hext2_c = [const.tile([P, 65], F16, tag=f"h2_{c}", name=f"h2_{c}")
                   for c in range(NJC)]
        dstL1 = const.tile([P, NJC], F32)    # dst/L - 1 (per-chunk columns)
        dstL1h = const.tile([P, NJC], F16)
        SL = const.tile([P, 1024], F16)      # src/L broadcast to 128 parts
        G_sb = const.tile([65, 1], F32)
        H_sb = const.tile([65, 1], F32)
        sig = [const.tile([P, 1], F32, tag=f"sg{q}", name=f"sg{q}") for q in range(8)]

        # ---- stage A ----
        XCH = 4
        XW = N // XCH
        with tc.tile_pool(name="prep", bufs=1) as prep, \
             tc.tile_pool(name="prep_ps", bufs=2, space="PSUM") as prep_ps:
            xt_t = [prep.tile([P, XW], BF16, tag=f"xta{k}", name=f"xta{k}")
                    for k in range(XCH)]
            xt_b = [prep.tile([P, XW], BF16, tag=f"xtb{k}", name=f"xtb{k}")
                    for k in range(XCH)]
            ht_sb = prep.tile([DOUT, N], F32)
            hext_c = [prep.tile([P, 65], F16, tag=f"hx{c}", name=f"hx{c}")
                      for c in range(NJC)]
            dstall = prep.tile([P, NJC], F32)
            B1 = prep.tile([P, NJC], F32)
            B2 = prep.tile([P, NJC], F32)

            pid = nc.partition_id()

            for k in range(XCH):
                nc.sync.dma_start(out=xt_t[k], in_=x_d[0:P, k * XW : (k + 1) * XW])
                nc.sync.dma_start(out=xt_b[k], in_=x_d[P:DIN, k * XW : (k + 1) * XW])

            # hT [64, N]; dst batches interleaved
            for m in range(N // 512):
                k, off = m // (XW // 512), (m % (XW // 512)) * 512
                ht_ps = prep_ps.tile([DOUT, 512], F32, tag="pp", name="ht_ps")
                nc.tensor.matmul(ht_ps, lhsT=w_a,
                                 rhs=xt_t[k][:, off : off + 512],
                                 start=True, stop=False)
                nc.tensor.matmul(ht_ps, lhsT=w_b,
                                 rhs=xt_b[k][:, off : off + 512],
                                 start=False, stop=True)
                if m % 2 == 0:
                    nc.vector.tensor_copy(ht_sb[:, m * 512 : (m + 1) * 512], ht_ps)
                else:
                    nc.scalar.copy(ht_sb[:, m * 512 : (m + 1) * 512], ht_ps)
                if m % 2 == 1:
                    b = (m - 1) // 2
                    d_ps = prep_ps.tile([P, 8], F32, tag="pp", name="d_ps")
                    for bb in range(8):
                        c = b * 8 + bb
                        nc.tensor.matmul(d_ps[:, bb : bb + 1],
                                         lhsT=ht_sb[:, c * P : (c + 1) * P], rhs=a2,
                                         start=True, stop=True)
                    nc.vector.tensor_copy(dstall[:, b * 8 : (b + 1) * 8], d_ps)
                    # h chunks -> hext tiles [128 i, 64] (+ ones col)
                    for c in range(b * 8, (b + 1) * 8):
                        kk, off2 = c // (XW // P), (c % (XW // P)) * P
                        h_ps = prep_ps.tile([P, DOUT], F32, tag="pp", name="h_ps")
                        nc.tensor.matmul(h_ps, lhsT=xt_t[kk][:, off2 : off2 + P],
                                         rhs=w_a, start=True, stop=False)
                        nc.tensor.matmul(h_ps, lhsT=xt_b[kk][:, off2 : off2 + P],
                                         rhs=w_b, start=False, stop=True)
                        if c % 2 == 0:
                            nc.scalar.copy(hext_c[c][:, 0:DOUT], h_ps)
                        else:
                            nc.vector.tensor_copy(hext_c[c][:, 0:DOUT], h_ps)
                        nc.vector.memset(hext_c[c][:, DOUT : DOUT + 1], 1.0)

            # global max of dst -> M (shift for exp range)
            rowmax = prep.tile([P, 1], F32)
            nc.vector.tensor_reduce(out=rowmax, in_=dstall, axis=mybir.AxisListType.X, op=mybir.AluOpType.max)
            rt_ps = prep_ps.tile([1, P], F32, tag="pp", name="rt_ps")
            nc.tensor.matmul(rt_ps, lhsT=rowmax, rhs=idf, start=True, stop=True)
            rowmaxT = prep.tile([1, P], F32)
            nc.vector.tensor_copy(rowmaxT, rt_ps)
            gmax = prep.tile([1, 1], F32)
            nc.vector.tensor_reduce(out=gmax, in_=rowmaxT, axis=mybir.AxisListType.X, op=mybir.AluOpType.max)
            mb_ps = prep_ps.tile([P, 1], F32, tag="pp", name="mb_ps")
            nc.tensor.matmul(mb_ps, lhsT=ones1, rhs=gmax, start=True, stop=True)
            Mb = prep.tile([P, 1], F32)
            nc.vector.tensor_copy(Mb, mb_ps)
            negM = prep.tile([P, 1], F32)
            nc.vector.tensor_scalar_mul(negM, Mb, -1.0)
            neg02M = prep.tile([P, 1], F32)
            nc.vector.tensor_scalar_mul(neg02M, Mb, -0.2)
            Mb08 = prep.tile([P, 1], F32)
            nc.vector.tensor_scalar_mul(Mb08, Mb, 0.8)

            nc.scalar.activation(B1, dstall, mybir.ActivationFunctionType.Exp,
                                 bias=negM, scale=1.0)
            nc.scalar.activation(B2, dstall, mybir.ActivationFunctionType.Exp,
                                 bias=neg02M, scale=0.2)
            nc.vector.tensor_scalar(out=dstL1, in0=dstall, scalar1=1.0 / L,
                                    scalar2=-1.0, op0=mybir.AluOpType.mult,
                                    op1=mybir.AluOpType.add)
            nc.vector.tensor_copy(dstL1h, dstL1)

            # hext1/hext2: scale (h|1) by B1/B2 per chunk (split ACT/DVE)
            for c in range(NJC):
                nc.scalar.activation(hext1_c[c], hext_c[c],
                                     mybir.ActivationFunctionType.Copy,
                                     scale=B1[:, c : c + 1])
                nc.vector.tensor_scalar_mul(hext2_c[c], hext_c[c], B2[:, c : c + 1])

            # G/H sums over P-chunks (for the t-agg affine correction)
            g_ps = prep_ps.tile([65, 1], F32, tag="pp", name="g_ps")
            h_ps2 = prep_ps.tile([65, 1], F32, tag="pp2", name="h_ps2")
            for ii, c in enumerate(pjcs):
                nc.tensor.matmul(g_ps, lhsT=hext2_c[c], rhs=onescol,
                                 start=(ii == 0), stop=(ii == len(pjcs) - 1))
                nc.tensor.matmul(h_ps2, lhsT=hext2_c[c], rhs=dstL1h[:, c : c + 1],
                                 start=(ii == 0), stop=(ii == len(pjcs) - 1))
            nc.vector.tensor_copy(G_sb, g_ps)
            nc.vector.tensor_copy(H_sb, h_ps2)

            # src for this core's rows; srcL row; SL broadcast; sigma
            src_my = prep.tile([1, R], F32)
            for ib in range(2):
                s_ps = prep_ps.tile([1, 512], F32, tag="pp", name="s_ps")
                nc.tensor.matmul(
                    s_ps, lhsT=a1,
                    rhs=ht_sb[0:DOUT, bass.ds(pid * R + ib * 512, 512)],
                    start=True, stop=True)
                nc.vector.tensor_copy(src_my[:, ib * 512 : (ib + 1) * 512], s_ps)
            srcL = prep.tile([1, R], F32)
            nc.vector.tensor_scalar_mul(srcL, src_my, 1.0 / L)
            sl_ps = prep_ps.tile([P, 1024], F32, tag="slp", name="sl_ps")
            for ib in range(2):
                nc.tensor.matmul(sl_ps[:, ib * 512 : (ib + 1) * 512], lhsT=ones1,
                                 rhs=srcL[:, ib * 512 : (ib + 1) * 512],
                                 start=True, stop=True)
            nc.vector.tensor_copy(SL, sl_ps)

            ones11 = prep.tile([1, 1], F32)
            nc.vector.memset(ones11, 1.0)
            for q in range(8):
                sg_ps = prep_ps.tile([P, 1], F32, tag="pp", name="sg_ps")
                nc.tensor.matmul(sg_ps, lhsT=src_my[:, q * P : (q + 1) * P],
                                 rhs=ones11, start=True, stop=True)
                nc.scalar.activation(sig[q], sg_ps,
                                     mybir.ActivationFunctionType.Exp,
                                     bias=Mb08, scale=0.8)

        # ---- stage B ----
        vpool = ctx.enter_context(tc.tile_pool(name="vp", bufs=VBUFS))
        mpool = ctx.enter_context(tc.tile_pool(name="mp", bufs=MBUFS))
        xpool = ctx.enter_context(tc.tile_pool(name="xp", bufs=XBUFS))
        agg1_pool = ctx.enter_context(tc.tile_pool(name="ag1", bufs=2, space="PSUM"))
        agg2_pool = ctx.enter_context(tc.tile_pool(name="ag2", bufs=2, space="PSUM"))
        agg1b_pool = ctx.enter_context(tc.tile_pool(name="ag1b", bufs=2, space="PSUM"))

        agg1 = [agg1_pool.tile([65, 512], F32, tag=f"a1_{ib}", name=f"a1_{ib}", bufs=1)
                for ib in range(2)]
        agg2 = [agg2_pool.tile([65, 512], F32, tag=f"a2_{ib}", name=f"a2_{ib}", bufs=1)
                for ib in range(2)]
        agg1b = [agg1b_pool.tile([65, 512], F32, tag=f"ab_{ib}", name=f"ab_{ib}", bufs=1)
                 for ib in range(2)]

        loop_cm = tc.For_i(0, rep_loop, 1) if rep_loop > 1 else nullcontext()
        with loop_cm:
            _stageB(nc, tc, adjT_d, vpool, mpool, xpool,
                    agg1, agg2, agg1b, SL, dstL1, hext1_c, hext2_c, pjcs)

        # ---- finalize ----
        with tc.tile_pool(name="fin", bufs=4) as fin, \
             tc.tile_pool(name="fin_ps", bufs=1, space="PSUM") as fin_ps:
            for ib in range(2):
                a1_sb = fin.tile([65, 512], F32, tag="a1sb")
                nc.vector.tensor_copy(a1_sb, agg1[ib])
                a2_sb = fin.tile([65, 512], F32, tag="a2sb")
                nc.vector.tensor_copy(a2_sb, agg2[ib])
                ab_sb = fin.tile([65, 512], F32, tag="absb")
                nc.scalar.copy(ab_sb, agg1b[ib])
                # remove affine part of t-aggs: corr = G*srcL + H
                corr = fin.tile([65, 512], F32, tag="corr")
                nc.vector.tensor_scalar(out=corr,
                                        in0=SL[0:65, ib * 512 : (ib + 1) * 512],
                                        scalar1=G_sb, scalar2=H_sb,
                                        op0=mybir.AluOpType.mult,
                                        op1=mybir.AluOpType.add)
                nc.vector.tensor_tensor(out=a2_sb, in0=a2_sb, in1=corr,
                                        op=mybir.AluOpType.subtract)
                nc.vector.tensor_tensor(out=a2_sb, in0=a2_sb, in1=ab_sb,
                                        op=mybir.AluOpType.subtract)
                for q in range(4):
                    g = ib * 4 + q
                    o1 = fin_ps.tile([P, 65], F32, tag="o1")
                    nc.tensor.matmul(o1, lhsT=a1_sb[:, q * P : (q + 1) * P],
                                     rhs=idf[0:65, 0:65], start=True, stop=True)
                    o2 = fin_ps.tile([P, 65], F32, tag="o2")
                    nc.tensor.matmul(o2, lhsT=a2_sb[:, q * P : (q + 1) * P],
                                     rhs=idf[0:65, 0:65], start=True, stop=True)
                    comb = fin.tile([P, 65], F32, tag="comb")
                    nc.vector.tensor_scalar_mul(comb, o1, sig[g])
                    nc.vector.tensor_tensor(out=comb, in0=comb, in1=o2,
                                            op=mybir.AluOpType.add)
                    rc = fin.tile([P, 1], F32, tag="rc")
                    nc.vector.reciprocal(rc, comb[:, DOUT : DOUT + 1])
                    hp = fin.tile([P, DOUT], F32, tag="hp")
                    nc.vector.tensor_scalar_mul(hp, comb[:, 0:DOUT], rc)
                    # elu = max(x,0) + exp(min(x,0)) - 1
                    ng = fin.tile([P, DOUT], F32, tag="ng")
                    nc.vector.tensor_scalar_min(ng, hp, 0.0)
                    ex = fin.tile([P, DOUT], F32, tag="ex")
                    nc.scalar.activation(ex, ng, mybir.ActivationFunctionType.Exp)
                    ps_ = fin.tile([P, DOUT], F32, tag="ps_")
                    nc.vector.tensor_scalar_max(ps_, hp, 0.0)
                    ob = fin.tile([P, DOUT], F32, tag="ob")
                    nc.vector.tensor_tensor(out=ob, in0=ex, in1=ps_,
                                            op=mybir.AluOpType.add)
                    nc.vector.tensor_scalar_add(ob, ob, -1.0)
                    nc.sync.dma_start(out=out_d[g * P : (g + 1) * P, :], in_=ob)


def _stageB(nc, tc, adjT_d, vpool, mpool, xpool,
            agg1, agg2, agg1b, SL, dstL1, hext1_c, hext2_c, pjcs):
    adjT_r = adjT_d.rearrange("(k p) i -> p k i", p=P)
    nd = 0
    VJ = VW_JC
    for vp in range(NJC // VJ):
        V = vpool.tile([P, VJ * 1024], F16, tag="V")
        for h2 in range(VJ):
            jc = VJ * vp + h2
            nc.vector.tensor_scalar(
                out=V[:, h2 * 1024 : (h2 + 1) * 1024], in0=SL,
                scalar1=dstL1[:, jc : jc + 1], scalar2=None,
                op0=mybir.AluOpType.add)
        nc.gpsimd.dma_start(
            out=V.rearrange("p (k i) -> p k i", k=VJ),
            in_=adjT_r[:, VJ * vp : VJ * vp + VJ, :],
            accum_op=mybir.AluOpType.add)
        for h2 in range(VJ):
            jc = VJ * vp + h2
            t = V[:, h2 * 1024 : (h2 + 1) * 1024]
            M1 = mpool.tile([P, 1024], F16, tag="M1")
            nc.vector.tensor_scalar(out=M1, in0=t, scalar1=0.0, scalar2=None,
                                    op0=mybir.AluOpType.is_gt)
            for ib in range(2):
                nc.tensor.matmul(agg1[ib], lhsT=hext1_c[jc],
                                 rhs=M1[:, ib * 512 : (ib + 1) * 512],
                                 start=(jc == 0), stop=(jc == NJC - 1))
            if _is_d(jc):
                X = xpool.tile([P, 1024], F16, tag="X")
                nc.vector.tensor_scalar(out=X, in0=t, scalar1=-0.5, scalar2=None,
                                        op0=mybir.AluOpType.is_ge)
                nd += 1
                if M2_POOL_MOD and nd % M2_POOL_MOD == 0:
                    nc.gpsimd.tensor_tensor(out=X, in0=X, in1=M1,
                                            op=mybir.AluOpType.subtract)
                else:
                    nc.vector.tensor_tensor(out=X, in0=X, in1=M1,
                                            op=mybir.AluOpType.subtract)
                for ib in range(2):
                    nc.tensor.matmul(agg2[ib], lhsT=hext2_c[jc],
                                     rhs=X[:, ib * 512 : (ib + 1) * 512],
                                     start=(jc == 0), stop=(jc == NJC - 1))
            else:
                for ib in range(2):
                    nc.tensor.matmul(agg1b[ib], lhsT=hext2_c[jc],
                                     rhs=M1[:, ib * 512 : (ib + 1) * 512],
                                     start=(jc == pjcs[0]), stop=(jc == pjcs[-1]))
                    nc.tensor.matmul(agg2[ib], lhsT=hext2_c[jc],
                                     rhs=t[:, ib * 512 : (ib + 1) * 512],
                                     start=(jc == 0), stop=(jc == NJC - 1))


def make_in_maps(inputs):
    import ml_dtypes
    xt = np.ascontiguousarray(
        np.asarray(inputs["input"], np.float32)[0].T).astype(ml_dtypes.bfloat16)
    adj = np.asarray(inputs["adj"], np.int32)
    w = np.asarray(inputs["w"], np.float32).astype(ml_dtypes.bfloat16)
    a = np.ascontiguousarray(
        np.asarray(inputs["a"], np.float32).reshape(2 * DOUT, 1))
    in_maps = []
    for c in range(NCORES):
        adjT = np.ascontiguousarray(adj[c * R : (c + 1) * R, :].T.astype(np.int8))
        in_maps.append({"adjT": adjT, "xt": xt, "w": w, "av": a})
    return in_maps


def kernel(**inputs) -> np.ndarray:
    nc = build_module()
    in_maps = make_in_maps(inputs)
    res = bass_utils.run_bass_kernel_spmd(nc, in_maps, core_ids=list(range(NCORES)))
    out = np.concatenate([res.results[c]["out"] for c in range(NCORES)], axis=0)
    return out.astype(np.float32)


if __name__ == "__main__":
    rng = np.random.default_rng(0)
    ins = {
        "input": rng.standard_normal((1, N, DIN)).astype(np.float32),
        "adj": rng.integers(0, 2, size=(N, N)).astype(np.int32),
        "w": rng.standard_normal((DIN, DOUT)).astype(np.float32) * 0.1,
        "a": rng.standard_normal((2 * DOUT, 1)).astype(np.float32) * 0.1,
    }
    o = kernel(**ins)
    print("kernel out", o.shape, o.dtype)
